# revision 24
# baseline (speedup 1.0000x reference)
"""DualHOILoss Trainium2 kernel (8 NeuronCores, pure data parallel over batch).

Math (per batch b, point p, object o):
    u = basis_p + delta_p,  w_o = o - m_b
    d2[p,o] = |u/s - w|^2 = u.(-2w/s) + |u|^2/s^2 + |w|^2
computed as ONE f32r matmul with K=5 rows: lhsT rows [ux,uy,uz,|u|^2,1],
rhs rows [-2wx/s,-2wy/s,-2wz/s,1/s^2,|w|^2] -> PSUM holds the COMPLETE d2.
Tiles run in pairs sharing one K=10 lhsT slice (rows 0:5 even tile, 5:10
odd; rhs has two zero-padded variants).

Per pair: PE writes 388 vert cols/tile into psB (emitted first); ACT
drains the psB pair -> SBUF c2 (INF-padded to 390); PE writes the other
390 cols/tile into per-tile psA banks; DVE tensor_tensor_scan (min,min)
folds psA against c2 into junk. Depths: psA 3 tiles, psB 2 pairs, c2 2
pairs -> DVE is the only steady-state bottleneck. Choir: host gathers the
selected anchor into ws rows (same 5-row semantics); Pool multiplies
uT*ws (bf16); per-pair selector matmuls accumulate d2sel [NT,128].

Every ACT function (Copy/Exp/Ln/Relu) lives in the one
natural_log_exp_and_others table: sqrt(x) is computed as exp(0.5*ln(x)) and
the clamp as Relu(x-eps), so there are NO mid-kernel table switches and all
tails run inline per batch. DMAs are consolidated (1 main + 1 misc per
batch); batch-0's f32r rounding copies run on then-idle DVE, the rest on
Pool (birverifier requires a non-DMA last writer for f32r operands).
Point tiling keeps the SBUF-natural map p = 32q + tau.
"""

import numpy as np

B, P, A, V = 16, 4096, 32, 778
NCORES = 8
BPC = B // NCORES      # batches per core
NT = P // 128          # 32 point tiles per batch
NPAIR = NT // 2        # 16 tile pairs per batch
N1 = 390               # vert cols scanned straight from PSUM (per tile)
N2 = V - N1            # vert cols drained to SBUF (388)
W = N1                 # scan width (c2 is INF-padded N2 -> W)
INF = 3.0e38
CHUNK = 4              # pair-chunks for f32r rounding copies
CW = NPAIR * 128 // CHUNK
MAINW2 = 2 * NPAIR * 128           # uT | ws packed columns

_CACHE = {}


def _build_program():
    import concourse.bacc as bacc
    import concourse.mybir as mybir
    from concourse import tile

    # The act-table-load pass greedily assigns each activation the FIRST
    # set containing its function (ln -> natural_log, which lacks exp),
    # thrashing tables. Every function this kernel uses (Copy/Exp/Ln/Relu)
    # lives in natural_log_exp_and_others; restrict the candidate map to it
    # (indices preserved) so exactly one table load is emitted.
    if not getattr(bacc, "_act_tables_patched", False):
        _orig_get_tables = bacc.get_activation_tables

        def _only_ln_exp(arch):
            tabs = _orig_get_tables(arch)
            return {name: (funcs if name == "natural_log_exp_and_others"
                           else set())
                    for name, funcs in tabs.items()}

        bacc.get_activation_tables = _only_ln_exp
        bacc._act_tables_patched = True

    f32 = mybir.dt.float32
    f32r = mybir.dt.float32r
    bf16 = mybir.dt.bfloat16
    AF = mybir.ActivationFunctionType
    ALU = mybir.AluOpType
    AX = mybir.AxisListType

    nc = bacc.Bacc(None, target_bir_lowering=False)

    rhsd_d = nc.dram_tensor("rhsd", [BPC, 10, 2 * V], f32,
                            kind="ExternalInput")
    main_d = nc.dram_tensor("main", [BPC, 10, NPAIR * 128], f32,
                            kind="ExternalInput")
    prod_d = nc.dram_tensor("prodd", [BPC, 10, NPAIR * 128], bf16,
                            kind="ExternalInput")
    misc_d = nc.dram_tensor("misc", [BPC, 128, NT + 128], f32,
                            kind="ExternalInput")
    out_d = nc.dram_tensor("partials", [1, 2], f32, kind="ExternalOutput")

    # consts: bf16 selmats [10, 512] + ones col; f32 eps biases
    import ml_dtypes
    cpb = np.zeros((128, 32 * NPAIR + 2), np.float32)
    for kp in range(NPAIR):
        cpb[0:5, 32 * kp + 2 * kp] = 1.0
        cpb[5:10, 32 * kp + 2 * kp + 1] = 1.0
    cpb[:, 32 * NPAIR] = 1.0
    cpackb_d = nc.inline_tensor(cpb.astype(ml_dtypes.bfloat16), "cpackb")
    cpf = np.zeros((128, 2), np.float32)
    cpf[:, 0] = -1.0e-12
    cpf[:, 1] = 1.0e-12
    cpackf_d = nc.inline_tensor(cpf, "cpackf")

    with tile.TileContext(nc) as tc:
        with (
            tc.tile_pool(name="sb", bufs=1) as sb,            # persistent
            tc.tile_pool(name="sbb", bufs=2) as sbb,          # per-batch
            tc.tile_pool(name="sbj", bufs=2) as sbj,          # junk per-batch
            tc.tile_pool(name="sbt", bufs=4) as sbt,          # small tail bufs
            tc.tile_pool(name="psA", bufs=3, space="PSUM") as psA,   # 3 banks
            tc.tile_pool(name="psB", bufs=2, space="PSUM") as psB,   # 4 banks
            tc.tile_pool(name="psS", bufs=1, space="PSUM") as psS,   # 1 bank
        ):
            cstf = sb.tile([128, 2], f32, tag="cstf")
            cstb = sb.tile([128, 32 * NPAIR + 2], bf16, tag="cstb")
            selmb = cstb[0:10, 0: 32 * NPAIR]
            ones128b = cstb[:, 32 * NPAIR: 32 * NPAIR + 1]
            ones32b = cstb[0:NT, 32 * NPAIR: 32 * NPAIR + 1]

            c2bufs = []
            for i in range(4):
                c2 = sb.tile([128, 2 * W], f32, tag=f"c2_{i}")
                nc.gpsimd.memset(c2[:, N2:W], INF)
                nc.gpsimd.memset(c2[:, W + N2:], INF)
                c2bufs.append(c2)

            # d2sel accumulator bank; free col regions hold the loss
            # colsum accumulators; also the PE-warmup dummy target (every
            # region it touches is later re-zeroed by a start=True matmul)
            d2selT = psS.tile([NT, 512], f32, tag="pss")

            uTs, rhss, prods, junks, hcs, adTs = [], [], [], [], [], []
            for b in range(BPC):
                # ---- per-batch load (rhs DMA first: shortest critical path)
                rh_s = sbb.tile([10, 2 * V], f32, tag="rh_s")
                nc.sync.dma_start(rh_s[:], rhsd_d[b])
                mn = sbb.tile([10, NPAIR * 128], f32, tag="mn")
                nc.sync.dma_start(mn[:], main_d[b])
                prod = sbb.tile([10, NPAIR * 128], bf16, tag="prod")
                nc.sync.dma_start(prod[:], prod_d[b])
                mi = sbb.tile([128, NT + 128], f32, tag="mi")
                nc.sync.dma_start(mi[:], misc_d[b])
                if b == 0:
                    nc.sync.dma_start(cstb[:], cpackb_d[:])
                    nc.sync.dma_start(cstf[:], cpackf_d[:])
                hcs.append(mi[:, 0:NT])
                adTs.append(mi[0:NT, NT: NT + 128])
                prods.append(prod)

                uT = sbb.tile([10, NPAIR * 128], f32r, tag="uT")
                rhs = sbb.tile([10, 2 * V], f32r, tag="rhs")
                uTs.append(uT)
                rhss.append(rhs)
                if b == 0:
                    # startup: DVE is idle; use its fast all-SBUF copies
                    nc.vector.tensor_copy(uT[:, 0:CW], mn[:, 0:CW])
                    nc.vector.tensor_copy(rhs[:, 0:V], rh_s[:, 0:V])
                    nc.vector.tensor_copy(rhs[:, V:2 * V], rh_s[:, V:2 * V])
                else:
                    nc.gpsimd.tensor_copy(rhs[:], rh_s[:])
                    nc.gpsimd.tensor_copy(uT[:, 0:CW], mn[:, 0:CW])
                for cch in range(1, CHUNK):
                    nc.gpsimd.tensor_copy(
                        uT[:, CW * cch: CW * (cch + 1)],
                        mn[:, CW * cch: CW * (cch + 1)])

            for b in range(BPC):
                uT, rhs, prod = uTs[b], rhss[b], prods[b]
                junk = sbj.tile([128, W * NT], f32, tag="junk")
                junks.append(junk)

                # ---------------- per-pair main loop ----------------
                for kp in range(NPAIR):
                    lhsT = uT[:, 128 * kp: 128 * (kp + 1)]
                    ptB = psB.tile([128, 1024], f32, tag="ptB")
                    c2 = c2bufs[kp % 4]
                    for j in range(2):
                        rv = rhs[:, V * j: V * j + V]
                        nc.tensor.matmul(ptB[:, 512 * j: 512 * j + N2],
                                         lhsT, rv[:, N1:V],
                                         start=True, stop=True)
                    if b == 0 and kp == 0:
                        # startup fast path: per-half drains so scan 0
                        # starts as soon as its own half lands
                        for j in range(2):
                            nc.scalar.activation(
                                c2[:, W * j: W * j + N2],
                                ptB[:, 512 * j: 512 * j + N2], AF.Copy)
                    else:
                        nc.scalar.activation(
                            c2[:].rearrange(
                                "p (j w) -> p j w", j=2)[:, :, 0:N2],
                            ptB[:].rearrange(
                                "p (j w) -> p j w", j=2)[:, :, 0:N2],
                            AF.Copy)
                    for j in range(2):
                        t = 2 * kp + j
                        rv = rhs[:, V * j: V * j + V]
                        ptA = psA.tile([128, 512], f32, tag="ptA")
                        nc.tensor.matmul(ptA[:, 0:N1], lhsT, rv[:, 0:N1],
                                         start=True, stop=True)
                        nc.vector.tensor_tensor_scan(
                            out=junk[:, W * t: W * (t + 1)],
                            data0=ptA[:, 0:N1],
                            data1=c2[:, W * j: W * j + W],
                            initial=INF, op0=ALU.min, op1=ALU.min)
                    nc.tensor.matmul(d2selT[:, 256 * b: 256 * b + 128],
                                     selmb[:, 32 * kp: 32 * (kp + 1)],
                                     prod[:, 128 * kp: 128 * (kp + 1)],
                                     start=(kp == 0), stop=(kp == NPAIR - 1))

            # -------- tails (deps let these overlap the other batch) ----
            for b in range(BPC):
                cont = sbt.tile([128, NT], f32, tag="cont")
                nc.scalar.activation(
                    cont[:],
                    junks[b][:].rearrange(
                        "p (t w) -> p t w", w=W)[:, :, W - 1],
                    AF.Exp, scale=-100.0)
                cdiff = sbt.tile([128, NT], f32, tag="cdiff")
                nc.gpsimd.tensor_tensor(cdiff[:], cont[:], hcs[b],
                                        op=ALU.subtract)
                csq = sbt.tile([128, NT], bf16, tag="csq")
                nc.gpsimd.tensor_tensor(csq[:], cdiff[:], cdiff[:],
                                        op=ALU.mult)
                # dsel = exp(0.5*ln(relu(d2sel - eps) + eps))
                dclamp = sbt.tile([NT, 128], f32, tag="dclamp")
                nc.scalar.activation(
                    dclamp[:], d2selT[:, 256 * b: 256 * b + 128],
                    AF.Relu, bias=cstf[0:NT, 0:1])
                # contact colsum reuses the (now consumed) d2sel region
                nc.tensor.matmul(d2selT[0:1, 256 * b: 256 * b + NT],
                                 ones128b, csq[:], start=True, stop=True)
                lnv = sbt.tile([NT, 128], f32, tag="lnv")
                nc.scalar.activation(
                    lnv[:], dclamp[:], AF.Ln,
                    bias=cstf[0:NT, 1:2])
                dsel = sbt.tile([NT, 128], f32, tag="dsel")
                nc.scalar.activation(dsel[:], lnv[:], AF.Exp, scale=0.5)
                ddiff = sbt.tile([NT, 128], f32, tag="ddiff")
                nc.gpsimd.tensor_tensor(ddiff[:], dsel[:], adTs[b],
                                        op=ALU.subtract)
                dsq = sbt.tile([NT, 128], bf16, tag="dsq")
                nc.gpsimd.tensor_tensor(dsq[:], ddiff[:], ddiff[:],
                                        op=ALU.mult)
                nc.tensor.matmul(d2selT[0:1, 256 * b + 128: 256 * b + 256],
                                 ones32b, dsq[:], start=True, stop=True)

            # ---------------- final reduction (straight from PSUM) -------
            # choir parts live at cols 128:256 & 384:512, contact at
            # 0:NT & 256:256+NT -> strided [1,2,*] views, one reduce each
            res = sb.tile([1, 2], f32, tag="res")
            half = d2selT[0:1, :].rearrange("p (j c) -> p j c", j=2)
            nc.vector.tensor_reduce(
                res[:, 0:1], half[:, :, 128:256], axis=AX.XY, op=ALU.add)
            nc.vector.tensor_reduce(
                res[:, 1:2], half[:, :, 0:NT], axis=AX.XY, op=ALU.add)
            nc.sync.dma_start(out_d[:], res[:])

    nc.compile()
    return nc


def _get_program():
    if "nc" not in _CACHE:
        _CACHE["nc"] = _build_program()
    return _CACHE["nc"]


def _host_pack(verts, anchors, choir, hand_contacts, bps_mean, s, basis):
    """Build per-core input maps. p = 32q + tau (partition q, tile tau)."""
    inv_s = np.float32(1.0) / s
    inv_s2 = inv_s * inv_s

    delta = choir[:, :, 1:4]
    u = basis[None, :, :] + delta                      # (B, P, 3)
    usq = np.einsum('bpd,bpd->bp', u, u)               # (B, P)

    idx = choir[:, :, 5].astype(np.int64)              # (B, P)
    asel = np.take_along_axis(anchors, idx[:, :, None], axis=1)  # (B, P, 3)

    def packT(vec3, sq, last):
        outm = np.empty((10, NPAIR * 128), np.float32)
        v = vec3.reshape(128, NT, 3)
        q = sq.reshape(128, NT)
        o = last.reshape(128, NT)
        for j in range(2):
            blk = np.concatenate(
                [np.transpose(v[:, j::2, :], (2, 1, 0)),
                 q[:, j::2].T[None, :, :],
                 o[:, j::2].T[None, :, :]], axis=0)
            outm[5 * j: 5 * j + 5] = blk.reshape(5, NPAIR * 128)
        return outm

    import ml_dtypes
    ones = np.ones(P, np.float32)
    main = np.zeros((B, 10, NPAIR * 128), np.float32)
    prodd = np.zeros((B, 10, NPAIR * 128), ml_dtypes.bfloat16)
    rhsd = np.zeros((B, 10, 2 * V), np.float32)
    misc = np.zeros((B, 128, NT + 128), np.float32)
    misc[:, :, 0:NT] = hand_contacts.reshape(B, 128, NT)
    misc[:, 0:NT, NT:] = choir[:, :, 4].reshape(B, 128, NT).transpose(0, 2, 1)

    for bb in range(B):
        m = bps_mean[bb].reshape(3)
        uTb = packT(u[bb], usq[bb], ones)
        main[bb] = uTb
        wa = asel[bb] - m[None, :]
        wsb = packT((-2.0 * inv_s) * wa,
                    np.full(P, inv_s2, np.float32),
                    np.einsum('pd,pd->p', wa, wa))
        prodd[bb] = (uTb * wsb).astype(ml_dtypes.bfloat16)
        wv = verts[bb] - m[None, :]
        vr = np.concatenate(
            [(-2.0 * inv_s) * wv,
             np.full((V, 1), inv_s2, np.float32),
             np.einsum('vd,vd->v', wv, wv)[:, None]], axis=1)  # (V, 5)
        rhsd[bb, 0:5, 0:V] = vr.T
        rhsd[bb, 5:10, V:2 * V] = vr.T

    in_maps = []
    for c in range(NCORES):
        lo = BPC * c
        in_maps.append({
            "rhsd": rhsd[lo:lo + BPC],
            "main": main[lo:lo + BPC],
            "prodd": prodd[lo:lo + BPC],
            "misc": misc[lo:lo + BPC],
        })
    return in_maps


def kernel(verts, anchors, choir, hand_contacts, bps_mean, bps_scalar,
           bps_basis, _trace=False):
    from concourse.bass_utils import run_bass_kernel_spmd

    verts = np.ascontiguousarray(np.asarray(verts, np.float32))
    anchors = np.ascontiguousarray(np.asarray(anchors, np.float32))
    choir = np.ascontiguousarray(np.asarray(choir, np.float32))
    hand_contacts = np.ascontiguousarray(np.asarray(hand_contacts, np.float32))
    bps_mean = np.ascontiguousarray(np.asarray(bps_mean, np.float32))
    s = np.float32(np.asarray(bps_scalar).reshape(()))
    basis = np.ascontiguousarray(np.asarray(bps_basis, np.float32))

    nc = _get_program()
    in_maps = _host_pack(verts, anchors, choir, hand_contacts, bps_mean,
                         s, basis)
    res = run_bass_kernel_spmd(nc, in_maps, list(range(NCORES)), trace=_trace)
    parts = np.stack([np.asarray(r["partials"], np.float64).reshape(2)
                      for r in res.results])
    choir_loss = parts[:, 0].sum() / (B * P)
    contact_loss = parts[:, 1].sum() / (B * P)
    out = (np.float32(choir_loss), np.float32(contact_loss))
    if _trace:
        return out, res
    return out


# revision 26
# speedup vs baseline: 1.1090x; 1.1090x over previous
"""DualHOILoss Trainium2 kernel (8 NeuronCores, pure data parallel over batch).

Math (per batch b, point p, object o):
    u = basis_p + delta_p,  w_o = o - m_b
    d2[p,o] = |u/s - w|^2 = u.(-2w/s) + |u|^2/s^2 + |w|^2
computed as ONE f32r matmul with K=5 rows: lhsT rows [ux,uy,uz,|u|^2,1],
rhs rows [-2wx/s,-2wy/s,-2wz/s,1/s^2,|w|^2] -> PSUM holds the COMPLETE d2.
Tiles run in pairs sharing one K=10 lhsT slice (rows 0:5 even tile, 5:10
odd; rhs has two zero-padded variants).

Per pair: PE writes 388 vert cols/tile into psB (emitted first); ACT
drains the psB pair -> SBUF c2 (INF-padded to 390); PE writes the other
390 cols/tile into per-tile psA banks; DVE tensor_tensor_scan (min,min)
folds psA against c2 into junk. Depths: psA 3 tiles, psB 2 pairs, c2 2
pairs -> DVE is the only steady-state bottleneck. Choir: host gathers the
selected anchor into ws rows (same 5-row semantics); Pool multiplies
uT*ws (bf16); per-pair selector matmuls accumulate d2sel [NT,128].

Every ACT function (Copy/Exp/Ln/Relu) lives in the one
natural_log_exp_and_others table: sqrt(x) is computed as exp(0.5*ln(x)) and
the clamp as Relu(x-eps), so there are NO mid-kernel table switches and all
tails run inline per batch. DMAs are consolidated (1 main + 1 misc per
batch); batch-0's f32r rounding copies run on then-idle DVE, the rest on
Pool (birverifier requires a non-DMA last writer for f32r operands).
Point tiling keeps the SBUF-natural map p = 32q + tau.
"""

import numpy as np

B, P, A, V = 16, 4096, 32, 778
NCORES = 8
BPC = B // NCORES      # batches per core
NT = P // 128          # 32 point tiles per batch
NPAIR = NT // 2        # 16 tile pairs per batch
N1 = 390               # vert cols scanned straight from PSUM (per tile)
N2 = V - N1            # vert cols drained to SBUF (388)
W = N1                 # scan width (c2 is INF-padded N2 -> W)
INF = 3.0e38
CHUNK = 4              # pair-chunks for f32r rounding copies
CW = NPAIR * 128 // CHUNK
MAINW2 = 2 * NPAIR * 128           # uT | ws packed columns

_CACHE = {}


def _build_program():
    import concourse.bacc as bacc
    import concourse.mybir as mybir
    from concourse import tile

    # The act-table-load pass greedily assigns each activation the FIRST
    # set containing its function (ln -> natural_log, which lacks exp),
    # thrashing tables. Every function this kernel uses (Copy/Exp/Ln/Relu)
    # lives in natural_log_exp_and_others; restrict the candidate map to it
    # (indices preserved) so exactly one table load is emitted.
    if not getattr(bacc, "_act_tables_patched", False):
        _orig_get_tables = bacc.get_activation_tables

        def _only_ln_exp(arch):
            tabs = _orig_get_tables(arch)
            return {name: (funcs if name == "natural_log_exp_and_others"
                           else set())
                    for name, funcs in tabs.items()}

        bacc.get_activation_tables = _only_ln_exp
        bacc._act_tables_patched = True

    f32 = mybir.dt.float32
    f32r = mybir.dt.float32r
    bf16 = mybir.dt.bfloat16
    AF = mybir.ActivationFunctionType
    ALU = mybir.AluOpType
    AX = mybir.AxisListType

    nc = bacc.Bacc(None, target_bir_lowering=False)

    rhsd_d = nc.dram_tensor("rhsd", [BPC, 10, 2 * V], f32,
                            kind="ExternalInput")
    main_d = nc.dram_tensor("main", [BPC, 10, NPAIR * 128], f32,
                            kind="ExternalInput")
    prod_d = nc.dram_tensor("prodd", [BPC, 10, NPAIR * 128], bf16,
                            kind="ExternalInput")
    misc_d = nc.dram_tensor("misc", [BPC, 128, NT + 128], f32,
                            kind="ExternalInput")
    out_d = nc.dram_tensor("partials", [1, 2], f32, kind="ExternalOutput")

    # consts: bf16 selmats [10, 512] + ones col; f32 eps biases
    import ml_dtypes
    cpb = np.zeros((128, 32 * NPAIR + 2), np.float32)
    for kp in range(NPAIR):
        cpb[0:5, 32 * kp + 2 * kp] = 1.0
        cpb[5:10, 32 * kp + 2 * kp + 1] = 1.0
    cpb[:, 32 * NPAIR] = 1.0
    cpackb_d = nc.inline_tensor(cpb.astype(ml_dtypes.bfloat16), "cpackb")
    cpf = np.zeros((128, 2), np.float32)
    cpf[:, 0] = -1.0e-12
    cpf[:, 1] = 1.0e-12
    cpackf_d = nc.inline_tensor(cpf, "cpackf")

    with tile.TileContext(nc) as tc:
        with (
            tc.tile_pool(name="sb", bufs=1) as sb,            # persistent
            tc.tile_pool(name="sbb", bufs=2) as sbb,          # per-batch
            tc.tile_pool(name="sbj", bufs=2) as sbj,          # junk per-batch
            tc.tile_pool(name="sbt", bufs=4) as sbt,          # small tail bufs
            tc.tile_pool(name="psA", bufs=3, space="PSUM") as psA,   # 3 banks
            tc.tile_pool(name="psB", bufs=2, space="PSUM") as psB,   # 4 banks
            tc.tile_pool(name="psS", bufs=1, space="PSUM") as psS,   # 1 bank
        ):
            cstf = sb.tile([128, 2], f32, tag="cstf")
            cstb = sb.tile([128, 32 * NPAIR + 2], bf16, tag="cstb")
            selmb = cstb[0:10, 0: 32 * NPAIR]
            ones128b = cstb[:, 32 * NPAIR: 32 * NPAIR + 1]
            ones32b = cstb[0:NT, 32 * NPAIR: 32 * NPAIR + 1]

            c2bufs = []
            for i in range(4):
                c2 = sb.tile([128, 2 * W], f32, tag=f"c2_{i}")
                nc.gpsimd.memset(c2[:, N2:W], INF)
                nc.gpsimd.memset(c2[:, W + N2:], INF)
                c2bufs.append(c2)

            # d2sel accumulator bank; free col regions hold the loss
            # colsum accumulators; also the PE-warmup dummy target (every
            # region it touches is later re-zeroed by a start=True matmul)
            d2selT = psS.tile([NT, 512], f32, tag="pss")

            warm = sb.tile([10, 512], bf16, tag="warm")
            nc.gpsimd.memset(warm[:], 0.0)
            for wv in range(8):
                nc.tensor.matmul(d2selT[0:2, 0:512], warm[:, 0:2],
                                 warm[:], start=True, stop=True)

            uTs, rhss, prods, junks, hcs, adTs = [], [], [], [], [], []
            for b in range(BPC):
                # ---- per-batch load (rhs DMA first: shortest critical path)
                rh_s = sbb.tile([10, 2 * V], f32, tag="rh_s")
                nc.sync.dma_start(rh_s[:], rhsd_d[b])
                mn = sbb.tile([10, NPAIR * 128], f32, tag="mn")
                nc.sync.dma_start(mn[:], main_d[b])
                prod = sbb.tile([10, NPAIR * 128], bf16, tag="prod")
                nc.sync.dma_start(prod[:], prod_d[b])
                mi = sbb.tile([128, NT + 128], f32, tag="mi")
                nc.sync.dma_start(mi[:], misc_d[b])
                if b == 0:
                    nc.sync.dma_start(cstb[:], cpackb_d[:])
                    nc.sync.dma_start(cstf[:], cpackf_d[:])
                hcs.append(mi[:, 0:NT])
                adTs.append(mi[0:NT, NT: NT + 128])
                prods.append(prod)

                uT = sbb.tile([10, NPAIR * 128], f32r, tag="uT")
                rhs = sbb.tile([10, 2 * V], f32r, tag="rhs")
                uTs.append(uT)
                rhss.append(rhs)
                if b == 0:
                    # startup: DVE is idle; use its fast all-SBUF copies
                    nc.vector.tensor_copy(uT[:, 0:CW], mn[:, 0:CW])
                    nc.vector.tensor_copy(rhs[:, 0:V], rh_s[:, 0:V])
                    nc.vector.tensor_copy(rhs[:, V:2 * V], rh_s[:, V:2 * V])
                else:
                    nc.gpsimd.tensor_copy(rhs[:], rh_s[:])
                    nc.gpsimd.tensor_copy(uT[:, 0:CW], mn[:, 0:CW])
                for cch in range(1, CHUNK):
                    nc.gpsimd.tensor_copy(
                        uT[:, CW * cch: CW * (cch + 1)],
                        mn[:, CW * cch: CW * (cch + 1)])

            for b in range(BPC):
                uT, rhs, prod = uTs[b], rhss[b], prods[b]
                junk = sbj.tile([128, W * NT], f32, tag="junk")
                junks.append(junk)

                # ---------------- per-pair main loop ----------------
                for kp in range(NPAIR):
                    lhsT = uT[:, 128 * kp: 128 * (kp + 1)]
                    ptB = psB.tile([128, 1024], f32, tag="ptB")
                    c2 = c2bufs[kp % 4]
                    for j in range(2):
                        rv = rhs[:, V * j: V * j + V]
                        nc.tensor.matmul(ptB[:, 512 * j: 512 * j + N2],
                                         lhsT, rv[:, N1:V],
                                         start=True, stop=True)
                    if b == 0 and kp == 0:
                        # startup fast path: per-half drains so scan 0
                        # starts as soon as its own half lands
                        for j in range(2):
                            nc.scalar.activation(
                                c2[:, W * j: W * j + N2],
                                ptB[:, 512 * j: 512 * j + N2], AF.Copy)
                    else:
                        nc.scalar.activation(
                            c2[:].rearrange(
                                "p (j w) -> p j w", j=2)[:, :, 0:N2],
                            ptB[:].rearrange(
                                "p (j w) -> p j w", j=2)[:, :, 0:N2],
                            AF.Copy)
                    for j in range(2):
                        t = 2 * kp + j
                        rv = rhs[:, V * j: V * j + V]
                        ptA = psA.tile([128, 512], f32, tag="ptA")
                        nc.tensor.matmul(ptA[:, 0:N1], lhsT, rv[:, 0:N1],
                                         start=True, stop=True)
                        nc.vector.tensor_tensor_scan(
                            out=junk[:, W * t: W * (t + 1)],
                            data0=ptA[:, 0:N1],
                            data1=c2[:, W * j: W * j + W],
                            initial=INF, op0=ALU.min, op1=ALU.min)
                    nc.tensor.matmul(d2selT[:, 256 * b: 256 * b + 128],
                                     selmb[:, 32 * kp: 32 * (kp + 1)],
                                     prod[:, 128 * kp: 128 * (kp + 1)],
                                     start=(kp == 0), stop=(kp == NPAIR - 1))

            # -------- tails (deps let these overlap the other batch) ----
            for b in range(BPC):
                cont = sbt.tile([128, NT], f32, tag="cont")
                nc.scalar.activation(
                    cont[:],
                    junks[b][:].rearrange(
                        "p (t w) -> p t w", w=W)[:, :, W - 1],
                    AF.Exp, scale=-100.0)
                cdiff = sbt.tile([128, NT], f32, tag="cdiff")
                csq = sbt.tile([128, NT], bf16, tag="csq")
                if b == BPC - 1:
                    # end chain: DVE is idle and its small ops are faster
                    nc.vector.tensor_tensor(cdiff[:], cont[:], hcs[b],
                                            op=ALU.subtract)
                    nc.vector.tensor_tensor(csq[:], cdiff[:], cdiff[:],
                                            op=ALU.mult)
                else:
                    nc.gpsimd.tensor_tensor(cdiff[:], cont[:], hcs[b],
                                            op=ALU.subtract)
                    nc.gpsimd.tensor_tensor(csq[:], cdiff[:], cdiff[:],
                                            op=ALU.mult)
                # dsel = exp(0.5*ln(relu(d2sel - eps) + eps))
                dclamp = sbt.tile([NT, 128], f32, tag="dclamp")
                nc.scalar.activation(
                    dclamp[:], d2selT[:, 256 * b: 256 * b + 128],
                    AF.Relu, bias=cstf[0:NT, 0:1])
                # contact colsum reuses the (now consumed) d2sel region
                nc.tensor.matmul(d2selT[0:1, 256 * b: 256 * b + NT],
                                 ones128b, csq[:], start=True, stop=True)
                lnv = sbt.tile([NT, 128], f32, tag="lnv")
                nc.scalar.activation(
                    lnv[:], dclamp[:], AF.Ln,
                    bias=cstf[0:NT, 1:2])
                dsel = sbt.tile([NT, 128], f32, tag="dsel")
                nc.scalar.activation(dsel[:], lnv[:], AF.Exp, scale=0.5)
                ddiff = sbt.tile([NT, 128], f32, tag="ddiff")
                nc.gpsimd.tensor_tensor(ddiff[:], dsel[:], adTs[b],
                                        op=ALU.subtract)
                dsq = sbt.tile([NT, 128], bf16, tag="dsq")
                nc.gpsimd.tensor_tensor(dsq[:], ddiff[:], ddiff[:],
                                        op=ALU.mult)
                nc.tensor.matmul(d2selT[0:1, 256 * b + 128: 256 * b + 256],
                                 ones32b, dsq[:], start=True, stop=True)

            # ---------------- final reduction (straight from PSUM) -------
            # choir parts live at cols 128:256 & 384:512, contact at
            # 0:NT & 256:256+NT -> strided [1,2,*] views, one reduce each
            res = sb.tile([1, 2], f32, tag="res")
            half = d2selT[0:1, :].rearrange("p (j c) -> p j c", j=2)
            nc.vector.tensor_reduce(
                res[:, 0:1], half[:, :, 128:256], axis=AX.XY, op=ALU.add)
            nc.vector.tensor_reduce(
                res[:, 1:2], half[:, :, 0:NT], axis=AX.XY, op=ALU.add)
            nc.sync.dma_start(out_d[:], res[:])

    nc.compile()
    return nc


def _get_program():
    if "nc" not in _CACHE:
        _CACHE["nc"] = _build_program()
    return _CACHE["nc"]


def _host_pack(verts, anchors, choir, hand_contacts, bps_mean, s, basis):
    """Build per-core input maps. p = 32q + tau (partition q, tile tau)."""
    inv_s = np.float32(1.0) / s
    inv_s2 = inv_s * inv_s

    delta = choir[:, :, 1:4]
    u = basis[None, :, :] + delta                      # (B, P, 3)
    usq = np.einsum('bpd,bpd->bp', u, u)               # (B, P)

    idx = choir[:, :, 5].astype(np.int64)              # (B, P)
    asel = np.take_along_axis(anchors, idx[:, :, None], axis=1)  # (B, P, 3)

    def packT(vec3, sq, last):
        outm = np.empty((10, NPAIR * 128), np.float32)
        v = vec3.reshape(128, NT, 3)
        q = sq.reshape(128, NT)
        o = last.reshape(128, NT)
        for j in range(2):
            blk = np.concatenate(
                [np.transpose(v[:, j::2, :], (2, 1, 0)),
                 q[:, j::2].T[None, :, :],
                 o[:, j::2].T[None, :, :]], axis=0)
            outm[5 * j: 5 * j + 5] = blk.reshape(5, NPAIR * 128)
        return outm

    import ml_dtypes
    ones = np.ones(P, np.float32)
    main = np.zeros((B, 10, NPAIR * 128), np.float32)
    prodd = np.zeros((B, 10, NPAIR * 128), ml_dtypes.bfloat16)
    rhsd = np.zeros((B, 10, 2 * V), np.float32)
    misc = np.zeros((B, 128, NT + 128), np.float32)
    misc[:, :, 0:NT] = hand_contacts.reshape(B, 128, NT)
    misc[:, 0:NT, NT:] = choir[:, :, 4].reshape(B, 128, NT).transpose(0, 2, 1)

    for bb in range(B):
        m = bps_mean[bb].reshape(3)
        uTb = packT(u[bb], usq[bb], ones)
        main[bb] = uTb
        wa = asel[bb] - m[None, :]
        wsb = packT((-2.0 * inv_s) * wa,
                    np.full(P, inv_s2, np.float32),
                    np.einsum('pd,pd->p', wa, wa))
        prodd[bb] = (uTb * wsb).astype(ml_dtypes.bfloat16)
        wv = verts[bb] - m[None, :]
        vr = np.concatenate(
            [(-2.0 * inv_s) * wv,
             np.full((V, 1), inv_s2, np.float32),
             np.einsum('vd,vd->v', wv, wv)[:, None]], axis=1)  # (V, 5)
        rhsd[bb, 0:5, 0:V] = vr.T
        rhsd[bb, 5:10, V:2 * V] = vr.T

    in_maps = []
    for c in range(NCORES):
        lo = BPC * c
        in_maps.append({
            "rhsd": rhsd[lo:lo + BPC],
            "main": main[lo:lo + BPC],
            "prodd": prodd[lo:lo + BPC],
            "misc": misc[lo:lo + BPC],
        })
    return in_maps


def kernel(verts, anchors, choir, hand_contacts, bps_mean, bps_scalar,
           bps_basis, _trace=False):
    from concourse.bass_utils import run_bass_kernel_spmd

    verts = np.ascontiguousarray(np.asarray(verts, np.float32))
    anchors = np.ascontiguousarray(np.asarray(anchors, np.float32))
    choir = np.ascontiguousarray(np.asarray(choir, np.float32))
    hand_contacts = np.ascontiguousarray(np.asarray(hand_contacts, np.float32))
    bps_mean = np.ascontiguousarray(np.asarray(bps_mean, np.float32))
    s = np.float32(np.asarray(bps_scalar).reshape(()))
    basis = np.ascontiguousarray(np.asarray(bps_basis, np.float32))

    nc = _get_program()
    in_maps = _host_pack(verts, anchors, choir, hand_contacts, bps_mean,
                         s, basis)
    res = run_bass_kernel_spmd(nc, in_maps, list(range(NCORES)), trace=_trace)
    parts = np.stack([np.asarray(r["partials"], np.float64).reshape(2)
                      for r in res.results])
    choir_loss = parts[:, 0].sum() / (B * P)
    contact_loss = parts[:, 1].sum() / (B * P)
    out = (np.float32(choir_loss), np.float32(contact_loss))
    if _trace:
        return out, res
    return out
